# revision 7
# baseline (speedup 1.0000x reference)
"""Trainium2 Bass kernel for the DiffPool-style GNN (nn_GNNpaper_75368086110729).

Strategy (data-parallel over graphs, 8 graphs/core on 8 cores):
  - Host: convert each graph's edge list to a padded per-source-row CSR
    (unique dst indices + duplicate counts) - pure index/layout prep.
  - Device, per graph:
      * build the dense [1024,1024] adjacency in SBUF as 8 fp16 tiles
        [128 src-partitions x 1024 dst] via the GPSIMD local_scatter ucode op
      * all sparse ops become dense matmuls with the adjacency as the
        PE moving operand: deg = 1^T a, agg1^T = x^T a, agg2^T = z^T a,
        u^T = s^T a (plus pooled = s^T z and G = s^T s riding the same pass)
      * diff-pool losses via  ||a - s s^T||^2 = sum(cnt^2) - 2 tr(adj_p) + ||s^T s||_F^2
      * dense SAGE + classifier on [64,128] tiles
  - Host: concat logits, combine the 4 scalar partial sums
    (loss = sqrt(link_sq)/a.size + ent/(B*N)).
"""

import numpy as np

B, N, E_PER = 64, 1024, 32768
IN_DIM, HID, K = 128, 128, 64
NCORES = 8
GPC = B // NCORES  # graphs per core
NG = N // 128      # 128-row groups per graph

_compiled = {}


def _prep_csr(edge_index):
    """edge_index [2, B*E_PER] int32 -> [B, NG, 2, 128, NI] int16 packed
    (t=0: unique dst idx, -1 pad; t=1: fp16 duplicate counts bit-cast)."""
    src = np.asarray(edge_index[0], dtype=np.int64)
    dst = np.asarray(edge_index[1], dtype=np.int64)
    g = src >> 10
    s = src & 1023
    d = dst & 1023
    key = (g << 20) | (s << 10) | d
    uk, cnts = np.unique(key, return_counts=True)
    rows = (uk >> 10).astype(np.int64)
    dloc = (uk & 1023).astype(np.int16)
    rowlen = np.bincount(rows, minlength=B * N)
    NI = int(max(2, ((rowlen.max() + 1) // 2) * 2))
    row_starts = np.zeros(B * N, np.int64)
    np.cumsum(rowlen[:-1], out=row_starts[1:])
    pos = np.arange(len(uk)) - row_starts[rows]
    idx = np.full((B * N, NI), -1, np.int16)
    cnt = np.zeros((B * N, NI), np.float16)
    idx[rows, pos] = dloc
    cnt[rows, pos] = cnts.astype(np.float16)
    packed = np.stack([idx.reshape(B, NG, 128, NI),
                       cnt.view(np.int16).reshape(B, NG, 128, NI)], axis=2)
    return np.ascontiguousarray(packed), NI


def _build(NI):
    import concourse.bacc as bacc
    import concourse.mybir as mybir
    from concourse import tile, library_config

    f16 = mybir.dt.float16
    f32 = mybir.dt.float32
    i16 = mybir.dt.int16
    AF = mybir.ActivationFunctionType
    ALU = mybir.AluOpType
    AX = mybir.AxisListType

    nc = bacc.Bacc("TRN2", target_bir_lowering=False, debug=False)

    def din(name, shape, dt=f32):
        return nc.dram_tensor(name, shape, dt, kind="ExternalInput").ap()

    xg = din("xg", [GPC * N, IN_DIM])
    idxcnt = din("idxcnt", [GPC, NG, 2, 128, NI], i16)
    ident_in = din("ident", [128, 128], f16)
    W1l_in = din("W1l", [IN_DIM, HID])
    W1r_in = din("W1r", [IN_DIM, HID])
    W2l_in = din("W2l", [HID, K])
    W2r_in = din("W2r", [HID, K])
    Wm_in = din("Wm", [K, K])
    Wrel_in = din("Wrel", [HID, HID])
    Wroot_in = din("Wroot", [HID, HID])
    Wc1_in = din("Wc1", [K * HID, HID])
    Wc2_in = din("Wc2", [HID, 1])
    b1l_in = din("b1l", [HID, 1])
    b2l_in = din("b2l", [K, 1])
    bm_in = din("bm", [K, 1])
    broot_in = din("broot", [HID, 1])
    bc1_in = din("bc1", [HID, 1])
    bc2_in = din("bc2", [1, 1])

    logits_out = nc.dram_tensor("logits", [1, GPC], f32, kind="ExternalOutput").ap()
    misc_out = nc.dram_tensor("misc", [4, 1], f32, kind="ExternalOutput").ap()

    with tile.TileContext(nc) as tc:
        nc.gpsimd.load_library(library_config.local_scatter)
        with (
            tc.tile_pool(name="const", bufs=1) as cpool,
            tc.tile_pool(name="a", bufs=2) as apool,
            tc.tile_pool(name="io", bufs=3) as iopool,
            tc.tile_pool(name="nodes", bufs=2) as npool,
            tc.tile_pool(name="small", bufs=2) as spool,
            tc.tile_pool(name="psb", bufs=3, space="PSUM") as psb,   # [128,512] f32
            tc.tile_pool(name="pss", bufs=3, space="PSUM") as pss,   # [64,512] f32
            tc.tile_pool(name="psm", bufs=2, space="PSUM") as psm,   # small f32
        ):
            # ---------------- constants / weights ----------------
            ident = cpool.tile([128, 128], f16)
            nc.sync.dma_start(ident[:], ident_in[:])
            ones_col = cpool.tile([128, 1], f16)
            nc.vector.memset(ones_col[:], 1.0)
            ones_row = cpool.tile([1, 128], f16)
            nc.vector.memset(ones_row[:], 1.0)
            ones32 = cpool.tile([128, 1], f32)
            nc.vector.memset(ones32[:], 1.0)

            def wtile(ap_in, p, q, name):
                t32 = iopool.tile([p, q], f32, tag="w32")
                nc.sync.dma_start(t32[:], ap_in[:])
                t = cpool.tile([p, q], f16, tag=name)
                nc.vector.tensor_copy(t[:], t32[:])
                return t

            W1l = wtile(W1l_in, 128, 128, "W1l")
            W1r = wtile(W1r_in, 128, 128, "W1r")
            W2l = wtile(W2l_in, 128, 64, "W2l")
            W2r = wtile(W2r_in, 128, 64, "W2r")
            Wm = wtile(Wm_in, 64, 64, "Wm")
            Wrel = wtile(Wrel_in, 128, 128, "Wrel")
            Wroot = wtile(Wroot_in, 128, 128, "Wroot")
            Wc2 = wtile(Wc2_in, 128, 1, "Wc2")

            Wc1 = []
            for k in range(K):
                t32 = iopool.tile([128, 128], f32, tag="wc132")
                nc.sync.dma_start(t32[:], Wc1_in[k * 128:(k + 1) * 128, :])
                t = cpool.tile([128, 128], f16, tag=f"wc1_{k}")
                nc.vector.tensor_copy(t[:], t32[:])
                Wc1.append(t)

            def btile(ap_in, p, name):
                t = cpool.tile([p, 1], f32, tag=name)
                nc.sync.dma_start(t[:], ap_in[:])
                return t

            b1l = btile(b1l_in, 128, "b1l")
            b2l = btile(b2l_in, 64, "b2l")
            bm = btile(bm_in, 64, "bm")
            broot = btile(broot_in, 128, "broot")
            bc1 = btile(bc1_in, 128, "bc1")
            bc2 = btile(bc2_in, 1, "bc2")

            # loss accumulators
            S_ent = cpool.tile([128, B], f32)
            S_cnt2 = cpool.tile([128, GPC], f32)
            S_diag = cpool.tile([128, GPC], f32)
            S_gsq = cpool.tile([128, GPC], f32)
            for t in (S_ent, S_cnt2, S_diag, S_gsq):
                nc.vector.memset(t[:], 0.0)
            H = cpool.tile([128, GPC * K], f16)  # col k*GPC+g
            Hr = H[:].rearrange("p (k g) -> p k g", g=GPC)

            # ---------------- per-graph pipeline ----------------
            for g in range(GPC):
                # CSR load (one DMA), adjacency build, sum(cnt^2)
                icb = iopool.tile([128, NG, 2, NI], i16, tag="icb")
                nc.sync.dma_start(icb[:], idxcnt[g].rearrange("q t p i -> p q t i"))
                a_t = []
                for q in range(NG):
                    at = apool.tile([128, N], f16, tag=f"a{q}")
                    nc.gpsimd.local_scatter(
                        at[:], icb[:, q, 1, :].bitcast(f16), icb[:, q, 0, :],
                        channels=128, num_elems=N, num_idxs=NI,
                    )
                    a_t.append(at)
                cscr = iopool.tile([128, NG * NI], f16, tag="cscr")
                nc.scalar.activation(
                    cscr[:].rearrange("p (q i) -> p q i", i=NI),
                    icb[:, :, 1, :].bitcast(f16), AF.Square,
                    accum_out=S_cnt2[:, g:g + 1],
                )

                # x load + cast + transposed copy
                x32 = iopool.tile([128, N], f32, tag="x32")
                nc.sync.dma_start(
                    x32[:].rearrange("p (q f) -> p q f", q=NG),
                    xg[g * N:(g + 1) * N, :].rearrange("(q p) f -> p q f", p=128))
                x16 = npool.tile([128, N], f16, tag="x16")
                nc.vector.tensor_copy(x16[:], x32[:])
                xT = npool.tile([128, N], f16, tag="xT")
                for q in range(NG):
                    nc.scalar.dma_start_transpose(
                        xT[:, q * 128:(q + 1) * 128], x16[:, q * 128:(q + 1) * 128])

                # deg = 1^T a ; recipb = 1/max(deg,1) broadcast to 128 partitions
                deg_row = npool.tile([1, N], f16, tag="degrow")
                for h in range(2):
                    pd = psb.tile([1, 512], f32, tag="pbig")
                    for q in range(NG):
                        nc.tensor.matmul(
                            pd[:], ones_col[:], a_t[q][:, h * 512:(h + 1) * 512],
                            start=(q == 0), stop=(q == NG - 1),
                        )
                    nc.vector.tensor_copy(deg_row[:, h * 512:(h + 1) * 512], pd[:])
                recipb = npool.tile([128, N], f32, tag="recipb")
                for h in range(2):
                    pb = psb.tile([128, 512], f32, tag="pbig")
                    nc.tensor.matmul(pb[:], ones_row[:], deg_row[:, h * 512:(h + 1) * 512])
                    nc.vector.tensor_scalar_max(recipb[:, h * 512:(h + 1) * 512], pb[:], 1.0)
                nc.vector.reciprocal_approx_fast(recipb[:], recipb[:])

                # pass 1: agg1^T = x^T a ; normalize
                aggn1T = npool.tile([128, N], f16, tag="aggn1")
                for h in range(2):
                    p1 = psb.tile([128, 512], f32, tag="pbig")
                    for q in range(NG):
                        nc.tensor.matmul(
                            p1[:], x16[:, q * 128:(q + 1) * 128],
                            a_t[q][:, h * 512:(h + 1) * 512],
                            start=(q == 0), stop=(q == NG - 1),
                        )
                    nc.vector.tensor_tensor(
                        out=aggn1T[:, h * 512:(h + 1) * 512], in0=p1[:],
                        in1=recipb[:, h * 512:(h + 1) * 512], op=ALU.mult,
                    )

                # z^T = lrelu(W1l^T aggn1T + W1r^T xT + b1l)
                zT = npool.tile([128, N], f16, tag="zT")
                for h in range(2):
                    pz = psb.tile([128, 512], f32, tag="pbig")
                    nc.tensor.matmul(pz[:], W1l[:], aggn1T[:, h * 512:(h + 1) * 512],
                                     start=True, stop=False)
                    nc.tensor.matmul(pz[:], W1r[:], xT[:, h * 512:(h + 1) * 512],
                                     start=False, stop=True)
                    nc.scalar.activation(zT[:, h * 512:(h + 1) * 512], pz[:],
                                         AF.Lrelu, bias=b1l[:], alpha=0.01)
                z_all = npool.tile([128, N], f16, tag="z_all")
                for q in range(NG):
                    nc.scalar.dma_start_transpose(
                        z_all[:, q * 128:(q + 1) * 128], zT[:, q * 128:(q + 1) * 128])

                # pass 2: agg2^T = z^T a ; normalize
                aggn2T = npool.tile([128, N], f16, tag="aggn2")
                for h in range(2):
                    p2 = psb.tile([128, 512], f32, tag="pbig")
                    for q in range(NG):
                        nc.tensor.matmul(
                            p2[:], z_all[:, q * 128:(q + 1) * 128],
                            a_t[q][:, h * 512:(h + 1) * 512],
                            start=(q == 0), stop=(q == NG - 1),
                        )
                    nc.vector.tensor_tensor(
                        out=aggn2T[:, h * 512:(h + 1) * 512], in0=p2[:],
                        in1=recipb[:, h * 512:(h + 1) * 512], op=ALU.mult,
                    )

                # s1^T = lrelu(W2l^T aggn2T + W2r^T zT + b2l); s2^T = Wm^T s1T + bm
                s1T = npool.tile([64, N], f16, tag="s1T")
                for h in range(2):
                    ps1 = pss.tile([64, 512], f32, tag="psmall")
                    nc.tensor.matmul(ps1[:], W2l[:], aggn2T[:, h * 512:(h + 1) * 512],
                                     start=True, stop=False)
                    nc.tensor.matmul(ps1[:], W2r[:], zT[:, h * 512:(h + 1) * 512],
                                     start=False, stop=True)
                    nc.scalar.activation(s1T[:, h * 512:(h + 1) * 512], ps1[:],
                                         AF.Lrelu, bias=b2l[:], alpha=0.01)
                s2T = npool.tile([64, N], f16, tag="s2T")
                for h in range(2):
                    ps2 = pss.tile([64, 512], f32, tag="psmall")
                    nc.tensor.matmul(ps2[:], Wm[:], s1T[:, h * 512:(h + 1) * 512])
                    nc.scalar.activation(s2T[:, h * 512:(h + 1) * 512], ps2[:],
                                         AF.Identity, bias=bm[:])
                s2_all = npool.tile([128, NG * K], f16, tag="s2_all")
                for q in range(NG):
                    nc.scalar.dma_start_transpose(
                        s2_all[:, q * K:(q + 1) * K], s2T[:, q * 128:(q + 1) * 128])

                # batched softmax over k + entropy pieces
                s2r = s2_all[:].rearrange("p (q k) -> p q k", k=K)
                Ms = spool.tile([128, NG], f32, tag="Ms")
                nc.vector.tensor_reduce(Ms[:], s2r, axis=AX.X, op=ALU.max)
                w_all = spool.tile([128, NG * K], f32, tag="w_all")
                nc.vector.tensor_tensor(
                    out=w_all[:].rearrange("p (q k) -> p q k", k=K), in0=s2r,
                    in1=Ms[:].unsqueeze(2).broadcast_to([128, NG, K]), op=ALU.subtract)
                t_all = npool.tile([128, NG * K], f16, tag="t_all")
                nc.scalar.activation(t_all[:], w_all[:], AF.Exp)
                Zs = spool.tile([128, NG], f32, tag="Zs")
                nc.vector.tensor_reduce(
                    Zs[:], t_all[:].rearrange("p (q k) -> p q k", k=K),
                    axis=AX.X, op=ALU.add)
                recipZ = spool.tile([128, NG], f32, tag="recipZ")
                nc.vector.reciprocal(recipZ[:], Zs[:])
                ss32 = spool.tile([128, NG * K], f32, tag="ss32")
                nc.vector.tensor_tensor(
                    out=ss32[:].rearrange("p (q k) -> p q k", k=K),
                    in0=t_all[:].rearrange("p (q k) -> p q k", k=K),
                    in1=recipZ[:].unsqueeze(2).broadcast_to([128, NG, K]), op=ALU.mult)
                ss_all = npool.tile([128, NG * K], f16, tag="ss_all")
                nc.vector.tensor_copy(ss_all[:], ss32[:])
                tws = spool.tile([128, NG * K], f32, tag="tws")
                nc.vector.tensor_tensor(out=tws[:], in0=t_all[:], in1=w_all[:],
                                        op=ALU.mult)
                Tw = spool.tile([128, NG], f32, tag="Tw")
                nc.vector.tensor_reduce(
                    Tw[:], tws[:].rearrange("p (q k) -> p q k", k=K),
                    axis=AX.X, op=ALU.add)
                lnZ = spool.tile([128, NG], f32, tag="lnZ")
                nc.scalar.activation(lnZ[:], Zs[:], AF.Ln)
                twz = spool.tile([128, NG], f32, tag="twz")
                nc.vector.tensor_tensor(out=twz[:], in0=Tw[:], in1=recipZ[:], op=ALU.mult)
                nc.vector.tensor_tensor(
                    out=S_ent[:, g * NG:(g + 1) * NG], in0=lnZ[:], in1=twz[:],
                    op=ALU.subtract)

                # pass 3: u^T = s^T a ; G = s^T s ; pooled = s^T z
                uT = npool.tile([64, N], f16, tag="uT")
                pG = psm.tile([64, 64], f32, tag="pm")
                ppool = psm.tile([64, 128], f32, tag="pm")
                for h in range(2):
                    pu = pss.tile([64, 512], f32, tag="psmall")
                    for q in range(NG):
                        nc.tensor.matmul(
                            pu[:], ss_all[:, q * K:(q + 1) * K],
                            a_t[q][:, h * 512:(h + 1) * 512],
                            start=(q == 0), stop=(q == NG - 1),
                        )
                    nc.vector.tensor_copy(uT[:, h * 512:(h + 1) * 512], pu[:])
                for q in range(NG):
                    nc.tensor.matmul(pG[:], ss_all[:, q * K:(q + 1) * K],
                                     ss_all[:, q * K:(q + 1) * K],
                                     start=(q == 0), stop=(q == NG - 1))
                for q in range(NG):
                    nc.tensor.matmul(ppool[:], ss_all[:, q * K:(q + 1) * K],
                                     z_all[:, q * 128:(q + 1) * 128],
                                     start=(q == 0), stop=(q == NG - 1))

                gscr = spool.tile([64, 64], f32, tag="gscr")
                nc.scalar.activation(gscr[:], pG[:], AF.Square,
                                     accum_out=S_gsq[:64, g:g + 1])
                pooled = spool.tile([64, 128], f16, tag="pooled")
                nc.vector.tensor_copy(pooled[:], ppool[:])

                u_all = npool.tile([128, NG * K], f16, tag="u_all")
                for q in range(NG):
                    nc.scalar.dma_start_transpose(
                        u_all[:, q * K:(q + 1) * K], uT[:, q * 128:(q + 1) * 128])
                padj = psm.tile([64, 64], f32, tag="pm")
                padjT = psm.tile([64, 64], f32, tag="pm")
                for q in range(NG):
                    nc.tensor.matmul(padj[:], u_all[:, q * K:(q + 1) * K],
                                     ss_all[:, q * K:(q + 1) * K],
                                     start=(q == 0), stop=(q == NG - 1))
                for q in range(NG):
                    nc.tensor.matmul(padjT[:], ss_all[:, q * K:(q + 1) * K],
                                     u_all[:, q * K:(q + 1) * K],
                                     start=(q == 0), stop=(q == NG - 1))

                dscr = spool.tile([64, 64], f32, tag="dscr")
                nc.vector.tensor_tensor(out=dscr[:], in0=padj[:], in1=ident[:64, :64],
                                        op=ALU.mult)
                nc.vector.tensor_reduce(S_diag[:64, g:g + 1], dscr[:], axis=AX.X,
                                        op=ALU.add)
                rs = spool.tile([64, 1], f32, tag="rs")
                nc.vector.tensor_reduce(rs[:], padj[:], axis=AX.X, op=ALU.add)
                nc.vector.tensor_scalar_max(rs[:], rs[:], 1.0)
                nc.vector.reciprocal(rs[:], rs[:])
                adjT = spool.tile([64, 64], f16, tag="adjT")
                nc.vector.tensor_copy(adjT[:], padjT[:])

                pagg3 = psm.tile([64, 128], f32, tag="pm")
                nc.tensor.matmul(pagg3[:], adjT[:], pooled[:])
                agg3n = spool.tile([64, 128], f16, tag="agg3n")
                nc.scalar.activation(agg3n[:], pagg3[:], AF.Copy, scale=rs[:])

                agg3nT = spool.tile([128, 64], f16, tag="agg3nT")
                nc.scalar.dma_start_transpose(agg3nT[:], agg3n[:])
                pooledT = spool.tile([128, 64], f16, tag="pooledT")
                nc.scalar.dma_start_transpose(pooledT[:], pooled[:])

                ph1 = psm.tile([128, 64], f32, tag="pm")
                nc.tensor.matmul(ph1[:], Wrel[:], agg3nT[:], start=True, stop=False)
                nc.tensor.matmul(ph1[:], Wroot[:], pooledT[:], start=False, stop=True)
                nc.scalar.activation(Hr[:, :, g], ph1[:], AF.Lrelu, bias=broot[:],
                                     alpha=0.01)

            # ---------------- classifier (all graphs) ----------------
            pcls = psm.tile([128, GPC], f32, tag="pm")
            for k in range(K):
                nc.tensor.matmul(pcls[:], Wc1[k][:], H[:, k * GPC:(k + 1) * GPC],
                                 start=(k == 0), stop=(k == K - 1))
            hc1 = spool.tile([128, GPC], f16, tag="hc1")
            nc.scalar.activation(hc1[:], pcls[:], AF.Lrelu, bias=bc1[:], alpha=0.01)
            plog = psm.tile([1, GPC], f32, tag="pm")
            nc.tensor.matmul(plog[:], Wc2[:], hc1[:])
            lsb = spool.tile([1, GPC], f32, tag="lsb")
            nc.scalar.activation(lsb[:], plog[:], AF.Identity, bias=bc2[:])
            nc.sync.dma_start(logits_out[:], lsb[:])

            # ---------------- loss partial sums ----------------
            Rall = spool.tile([128, 4], f32, tag="Rall")
            nc.vector.tensor_reduce(Rall[:, 0:1], S_ent[:], axis=AX.X, op=ALU.add)
            nc.vector.tensor_reduce(Rall[:, 1:2], S_cnt2[:], axis=AX.X, op=ALU.add)
            nc.vector.tensor_reduce(Rall[:, 2:3], S_diag[:], axis=AX.X, op=ALU.add)
            nc.vector.tensor_reduce(Rall[:, 3:4], S_gsq[:], axis=AX.X, op=ALU.add)
            pfin = psm.tile([4, 1], f32, tag="pm")
            nc.tensor.matmul(pfin[:], Rall[:], ones32[:])
            msb = spool.tile([4, 1], f32, tag="msb")
            nc.vector.tensor_copy(msb[:], pfin[:])
            nc.sync.dma_start(misc_out[:], msb[:])

    nc.compile()
    return nc


def kernel(x, edge_index, batch, num_graphs, nodes_per_graph,
           W1l, b1l, W1r, W2l, b2l, W2r, Wm, bm,
           Wrel, Wroot, broot, Wc1, bc1, Wc2, bc2, **_unused):
    import os
    from concourse.bass_utils import run_bass_kernel_spmd

    x = np.asarray(x, np.float32)
    idxcnt, NI = _prep_csr(np.asarray(edge_index))

    if NI not in _compiled:
        _compiled[NI] = _build(NI)
    nc = _compiled[NI]

    def f32c(a, shape=None):
        a = np.ascontiguousarray(np.asarray(a, np.float32))
        return a.reshape(shape) if shape is not None else a

    weights = dict(
        ident=np.eye(128, dtype=np.float16),
        W1l=f32c(W1l), W1r=f32c(W1r), W2l=f32c(W2l), W2r=f32c(W2r),
        Wm=f32c(Wm), Wrel=f32c(Wrel), Wroot=f32c(Wroot), Wc1=f32c(Wc1),
        Wc2=f32c(Wc2, (HID, 1)),
        b1l=f32c(b1l, (HID, 1)), b2l=f32c(b2l, (K, 1)), bm=f32c(bm, (K, 1)),
        broot=f32c(broot, (HID, 1)), bc1=f32c(bc1, (HID, 1)),
        bc2=f32c(bc2, (1, 1)),
    )
    in_maps = []
    for c in range(NCORES):
        in_maps.append(dict(
            xg=np.ascontiguousarray(x[c * GPC * N:(c + 1) * GPC * N]),
            idxcnt=idxcnt[c * GPC:(c + 1) * GPC],
            **weights,
        ))

    trace = bool(os.environ.get("KERNEL_TRACE"))
    res = run_bass_kernel_spmd(nc, in_maps, list(range(NCORES)), trace=trace)
    kernel.last_results = res

    logits = np.concatenate([res.results[c]["logits"][0] for c in range(NCORES)])
    ent = sum(float(res.results[c]["misc"][0, 0]) for c in range(NCORES))
    cnt2 = sum(float(res.results[c]["misc"][1, 0]) for c in range(NCORES))
    diag = sum(float(res.results[c]["misc"][2, 0]) for c in range(NCORES))
    gsq = sum(float(res.results[c]["misc"][3, 0]) for c in range(NCORES))
    link_sq = max(cnt2 - 2.0 * diag + gsq, 0.0)
    loss = np.float32(np.sqrt(link_sq) / (B * N * N) + ent / (B * N))
    return logits.astype(np.float32), loss


# revision 13
# speedup vs baseline: 1.3993x; 1.3993x over previous
"""Trainium2 Bass kernel for the DiffPool-style GNN (nn_GNNpaper_75368086110729).

Strategy (data-parallel over graphs, 8 graphs/core on 8 cores):
  - Host: convert each graph's edge list to a padded per-source-row CSR
    (unique dst indices + duplicate counts) - pure index/layout prep.
  - Device, per graph:
      * build the dense [1024,1024] adjacency in SBUF as 8 fp16 tiles
        [128 src-partitions x 1024 dst] via the GPSIMD local_scatter ucode op
      * all sparse ops become dense matmuls with the adjacency as the
        PE moving operand: deg = 1^T a, agg1^T = x^T a, agg2^T = z^T a,
        u^T = s^T a (plus pooled = s^T z and G = s^T s riding the same pass)
      * diff-pool losses via  ||a - s s^T||^2 = sum(cnt^2) - 2 tr(adj_p) + ||s^T s||_F^2
      * dense SAGE + classifier on [64,128] tiles
  - Host: concat logits, combine the 4 scalar partial sums
    (loss = sqrt(link_sq)/a.size + ent/(B*N)).
"""

import numpy as np

B, N, E_PER = 64, 1024, 32768
IN_DIM, HID, K = 128, 128, 64
NCORES = 8
GPC = B // NCORES  # graphs per core
NG = N // 128      # 128-row groups per graph

_compiled = {}


def _prep_csr(edge_index):
    """edge_index [2, B*E_PER] int32 -> [B, NG, 2, 128, NI] int16 packed
    (t=0: unique dst idx, -1 pad; t=1: fp16 duplicate counts bit-cast)."""
    src = np.asarray(edge_index[0], dtype=np.int64)
    dst = np.asarray(edge_index[1], dtype=np.int64)
    g = src >> 10
    s = src & 1023
    d = dst & 1023
    key = (g << 20) | (s << 10) | d
    uk, cnts = np.unique(key, return_counts=True)
    rows = (uk >> 10).astype(np.int64)
    dloc = (uk & 1023).astype(np.int16)
    rowlen = np.bincount(rows, minlength=B * N)
    NI = int(max(2, ((rowlen.max() + 1) // 2) * 2))
    row_starts = np.zeros(B * N, np.int64)
    np.cumsum(rowlen[:-1], out=row_starts[1:])
    pos = np.arange(len(uk)) - row_starts[rows]
    idx = np.full((B * N, NI), -1, np.int16)
    cnt = np.zeros((B * N, NI), np.float16)
    idx[rows, pos] = dloc
    cnt[rows, pos] = cnts.astype(np.float16)
    packed = np.stack([idx.reshape(B, NG, 128, NI),
                       cnt.view(np.int16).reshape(B, NG, 128, NI)], axis=2)

    # CSC-ordered counts (per-dst rows) - lets the device compute in-degrees
    # as cheap free-dim row sums in the transposed [128, NG] layout.
    key2 = (g << 20) | (d << 10) | s
    uk2, cnts2 = np.unique(key2, return_counts=True)
    rows2 = (uk2 >> 10).astype(np.int64)
    rowlen2 = np.bincount(rows2, minlength=B * N)
    NJ = int(max(2, ((rowlen2.max() + 1) // 2) * 2))
    row_starts2 = np.zeros(B * N, np.int64)
    np.cumsum(rowlen2[:-1], out=row_starts2[1:])
    pos2 = np.arange(len(uk2)) - row_starts2[rows2]
    cntcsc = np.zeros((B * N, NJ), np.float16)
    cntcsc[rows2, pos2] = cnts2.astype(np.float16)
    cntcsc = np.ascontiguousarray(cntcsc.reshape(B, NG, 128, NJ))
    return np.ascontiguousarray(packed), cntcsc, NI, NJ


def _build(NI, NJ):
    import concourse.bacc as bacc
    import concourse.mybir as mybir
    from concourse import tile, library_config

    f16 = mybir.dt.float16
    f32 = mybir.dt.float32
    i16 = mybir.dt.int16
    AF = mybir.ActivationFunctionType
    ALU = mybir.AluOpType
    AX = mybir.AxisListType

    nc = bacc.Bacc("TRN2", target_bir_lowering=False, debug=False)

    def din(name, shape, dt=f32):
        return nc.dram_tensor(name, shape, dt, kind="ExternalInput").ap()

    xg = din("xg", [GPC * N, IN_DIM])
    idxcnt = din("idxcnt", [GPC, NG, 2, 128, NI], i16)
    cntcsc_in = din("cntcsc", [GPC, NG, 128, NJ], f16)
    ident_in = din("ident", [128, 128], f16)
    W1l_in = din("W1l", [IN_DIM, HID])
    W1r_in = din("W1r", [IN_DIM, HID])
    W2l_in = din("W2l", [HID, K])
    W2r_in = din("W2r", [HID, K])
    Wm_in = din("Wm", [K, K])
    Wrel_in = din("Wrel", [HID, HID])
    Wroot_in = din("Wroot", [HID, HID])
    Wc1_in = din("Wc1", [K * HID, HID])
    Wc2_in = din("Wc2", [HID, 1])
    b1l_in = din("b1l", [HID, 1])
    b2l_in = din("b2l", [K, 1])
    bm_in = din("bm", [K, 1])
    broot_in = din("broot", [HID, 1])
    bc1_in = din("bc1", [HID, 1])
    bc2_in = din("bc2", [1, 1])

    logits_out = nc.dram_tensor("logits", [1, GPC], f32, kind="ExternalOutput").ap()
    misc_out = nc.dram_tensor("misc", [4, 1], f32, kind="ExternalOutput").ap()

    with tile.TileContext(nc) as tc:
        nc.gpsimd.load_library(library_config.local_scatter)
        with (
            tc.tile_pool(name="const", bufs=1) as cpool,
            tc.tile_pool(name="a", bufs=2) as apool,
            tc.tile_pool(name="io", bufs=3) as iopool,
            tc.tile_pool(name="nodes", bufs=2) as npool,
            tc.tile_pool(name="small", bufs=2) as spool,
            tc.tile_pool(name="psb", bufs=2, space="PSUM") as psb,   # [128,512] f32
            tc.tile_pool(name="pss", bufs=2, space="PSUM") as pss,   # [64,512] f32
            tc.tile_pool(name="pst", bufs=2, space="PSUM") as pst,   # [128,128] f16
            tc.tile_pool(name="psm", bufs=2, space="PSUM") as psm,   # small f32
        ):
            # ---------------- constants / weights ----------------
            ident = cpool.tile([128, 128], f16)
            nc.sync.dma_start(ident[:], ident_in[:])
            ones_col = cpool.tile([128, 1], f16)
            nc.vector.memset(ones_col[:], 1.0)
            ones_row = cpool.tile([1, 128], f16)
            nc.vector.memset(ones_row[:], 1.0)
            ones32 = cpool.tile([128, 1], f32)
            nc.vector.memset(ones32[:], 1.0)

            def wtile(ap_in, p, q, name):
                t32 = iopool.tile([p, q], f32, tag="w32")
                nc.sync.dma_start(t32[:], ap_in[:])
                t = cpool.tile([p, q], f16, tag=name)
                nc.vector.tensor_copy(t[:], t32[:])
                return t

            W1l = wtile(W1l_in, 128, 128, "W1l")
            W1r = wtile(W1r_in, 128, 128, "W1r")
            W2l = wtile(W2l_in, 128, 64, "W2l")
            W2r = wtile(W2r_in, 128, 64, "W2r")
            Wm = wtile(Wm_in, 64, 64, "Wm")
            Wrel = wtile(Wrel_in, 128, 128, "Wrel")
            Wroot = wtile(Wroot_in, 128, 128, "Wroot")
            Wc2 = wtile(Wc2_in, 128, 1, "Wc2")

            Wc1 = []
            for k in range(K):
                t32 = iopool.tile([128, 128], f32, tag="wc132")
                nc.sync.dma_start(t32[:], Wc1_in[k * 128:(k + 1) * 128, :])
                t = cpool.tile([128, 128], f16, tag=f"wc1_{k}")
                nc.vector.tensor_copy(t[:], t32[:])
                Wc1.append(t)

            def btile(ap_in, p, name):
                t = cpool.tile([p, 1], f32, tag=name)
                nc.sync.dma_start(t[:], ap_in[:])
                return t

            b1l = btile(b1l_in, 128, "b1l")
            b2l = btile(b2l_in, 64, "b2l")
            bm = btile(bm_in, 64, "bm")
            broot = btile(broot_in, 128, "broot")
            bc1 = btile(bc1_in, 128, "bc1")
            bc2 = btile(bc2_in, 1, "bc2")

            # loss accumulators
            S_ent = cpool.tile([128, B], f32)
            S_cnt2 = cpool.tile([128, GPC], f32)
            S_diag = cpool.tile([128, GPC], f32)
            S_gsq = cpool.tile([128, GPC], f32)
            for t in (S_ent, S_cnt2, S_diag, S_gsq):
                nc.vector.memset(t[:], 0.0)
            H = cpool.tile([128, GPC * K], f16)  # col k*GPC+g
            Hr = H[:].rearrange("p (k g) -> p k g", g=GPC)

            # ---------------- per-graph pipeline ----------------
            for g in range(GPC):
                # CSR load (one DMA), adjacency build, sum(cnt^2)
                icb = iopool.tile([128, NG, 2, NI], i16, tag="icb")
                nc.sync.dma_start(icb[:], idxcnt[g].rearrange("q t p i -> p q t i"))
                a_t = []
                for q in range(NG):
                    at = apool.tile([128, N], f16, tag=f"a{q}")
                    nc.gpsimd.local_scatter(
                        at[:], icb[:, q, 1, :].bitcast(f16), icb[:, q, 0, :],
                        channels=128, num_elems=N, num_idxs=NI,
                    )
                    a_t.append(at)
                cscr = iopool.tile([128, NG * NI], f16, tag="cscr")
                nc.scalar.activation(
                    cscr[:].rearrange("p (q i) -> p q i", i=NI),
                    icb[:, :, 1, :].bitcast(f16), AF.Square,
                    accum_out=S_cnt2[:, g:g + 1],
                )

                # x load + cast + transposed copy
                x32 = iopool.tile([128, N], f32, tag="x32")
                nc.sync.dma_start(
                    x32[:].rearrange("p (q f) -> p q f", q=NG),
                    xg[g * N:(g + 1) * N, :].rearrange("(q p) f -> p q f", p=128))
                x16 = npool.tile([128, N], f16, tag="x16")
                nc.vector.tensor_copy(x16[:], x32[:])
                xT = npool.tile([128, N], f16, tag="xT")
                for q in range(NG):
                    pt = pst.tile([128, 128], f16, tag="tp")
                    nc.tensor.transpose(pt[:], x16[:, q * 128:(q + 1) * 128], ident[:])
                    nc.vector.tensor_copy(xT[:, q * 128:(q + 1) * 128], pt[:])

                # deg from CSC count row-sums -> [128, NG] (partition = dst%128)
                ccs = iopool.tile([128, NG * NJ], f16, tag="ccs")
                nc.sync.dma_start(
                    ccs[:].rearrange("p (q j) -> p q j", j=NJ),
                    cntcsc_in[g].rearrange("q p j -> p q j"))
                deg_t = spool.tile([128, NG], f32, tag="deg_t")
                nc.vector.tensor_reduce(
                    deg_t[:], ccs[:].rearrange("p (q j) -> p q j", j=NJ),
                    axis=AX.X, op=ALU.add)
                nc.vector.tensor_scalar_max(deg_t[:], deg_t[:], 1.0)
                rcp_t = spool.tile([128, NG], f32, tag="rcp_t")
                nc.vector.reciprocal_approx_fast(rcp_t[:], deg_t[:])
                rcp16 = spool.tile([128, NG], f16, tag="rcp16")
                nc.vector.tensor_copy(rcp16[:], rcp_t[:])
                ptr = pst.tile([128, 128], f16, tag="tp")
                nc.tensor.transpose(ptr[:NG, :], rcp16[:], ident[:])
                rcpT = spool.tile([NG, 128], f16, tag="rcpT")
                nc.vector.tensor_copy(rcpT[:], ptr[:NG, :])
                recipb = npool.tile([128, N], f16, tag="recipb")
                for q in range(NG):
                    rq = spool.tile([1, 128], f16, tag=f"rcprow{q}")
                    nc.sync.dma_start(rq[:], rcpT[q:q + 1, :])
                    pb = pst.tile([128, 128], f32, tag="tp")
                    nc.tensor.matmul(pb[:], ones_row[:], rq[:])
                    nc.vector.tensor_copy(recipb[:, q * 128:(q + 1) * 128], pb[:])

                # pass 1: agg1^T = x^T a ; normalize
                aggn1T = npool.tile([128, N], f16, tag="aggn1")
                for h in range(2):
                    p1 = psb.tile([128, 512], f32, tag="pbig")
                    for q in range(NG):
                        nc.tensor.matmul(
                            p1[:], x16[:, q * 128:(q + 1) * 128],
                            a_t[q][:, h * 512:(h + 1) * 512],
                            start=(q == 0), stop=(q == NG - 1),
                        )
                    nc.vector.tensor_tensor(
                        out=aggn1T[:, h * 512:(h + 1) * 512], in0=p1[:],
                        in1=recipb[:, h * 512:(h + 1) * 512], op=ALU.mult,
                    )

                # z^T = lrelu(W1l^T aggn1T + W1r^T xT + b1l)
                zT = npool.tile([128, N], f16, tag="zT")
                for h in range(2):
                    pz = psb.tile([128, 512], f32, tag="pbig")
                    nc.tensor.matmul(pz[:], W1l[:], aggn1T[:, h * 512:(h + 1) * 512],
                                     start=True, stop=False)
                    nc.tensor.matmul(pz[:], W1r[:], xT[:, h * 512:(h + 1) * 512],
                                     start=False, stop=True)
                    nc.scalar.activation(zT[:, h * 512:(h + 1) * 512], pz[:],
                                         AF.Lrelu, bias=b1l[:], alpha=0.01)
                z_all = npool.tile([128, N], f16, tag="z_all")
                for q in range(NG):
                    pt = pst.tile([128, 128], f16, tag="tp")
                    nc.tensor.transpose(pt[:], zT[:, q * 128:(q + 1) * 128], ident[:])
                    nc.vector.tensor_copy(z_all[:, q * 128:(q + 1) * 128], pt[:])

                # pass 2: agg2^T = z^T a ; normalize
                aggn2T = npool.tile([128, N], f16, tag="aggn2")
                for h in range(2):
                    p2 = psb.tile([128, 512], f32, tag="pbig")
                    for q in range(NG):
                        nc.tensor.matmul(
                            p2[:], z_all[:, q * 128:(q + 1) * 128],
                            a_t[q][:, h * 512:(h + 1) * 512],
                            start=(q == 0), stop=(q == NG - 1),
                        )
                    nc.vector.tensor_tensor(
                        out=aggn2T[:, h * 512:(h + 1) * 512], in0=p2[:],
                        in1=recipb[:, h * 512:(h + 1) * 512], op=ALU.mult,
                    )

                # s1^T = lrelu(W2l^T aggn2T + W2r^T zT + b2l); s2^T = Wm^T s1T + bm
                s1T = npool.tile([64, N], f16, tag="s1T")
                for h in range(2):
                    ps1 = pss.tile([64, 512], f32, tag="psmall")
                    nc.tensor.matmul(ps1[:], W2l[:], aggn2T[:, h * 512:(h + 1) * 512],
                                     start=True, stop=False)
                    nc.tensor.matmul(ps1[:], W2r[:], zT[:, h * 512:(h + 1) * 512],
                                     start=False, stop=True)
                    nc.scalar.activation(s1T[:, h * 512:(h + 1) * 512], ps1[:],
                                         AF.Lrelu, bias=b2l[:], alpha=0.01)
                s2T = npool.tile([64, N], f16, tag="s2T")
                for h in range(2):
                    ps2 = pss.tile([64, 512], f32, tag="psmall")
                    nc.tensor.matmul(ps2[:], Wm[:], s1T[:, h * 512:(h + 1) * 512])
                    nc.scalar.activation(s2T[:, h * 512:(h + 1) * 512], ps2[:],
                                         AF.Identity, bias=bm[:])
                s2_all = npool.tile([128, NG * K], f16, tag="s2_all")
                for q in range(NG):
                    pt = pst.tile([128, 128], f16, tag="tp")
                    nc.tensor.transpose(pt[:, :K], s2T[:, q * 128:(q + 1) * 128],
                                        ident[:64, :64])
                    nc.scalar.copy(s2_all[:, q * K:(q + 1) * K], pt[:, :K])

                # batched softmax over k + entropy pieces
                s2r = s2_all[:].rearrange("p (q k) -> p q k", k=K)
                Ms = spool.tile([128, NG], f32, tag="Ms")
                nc.vector.tensor_reduce(Ms[:], s2r, axis=AX.X, op=ALU.max)
                w_all = spool.tile([128, NG * K], f32, tag="w_all")
                nc.vector.tensor_tensor(
                    out=w_all[:].rearrange("p (q k) -> p q k", k=K), in0=s2r,
                    in1=Ms[:].unsqueeze(2).broadcast_to([128, NG, K]), op=ALU.subtract)
                t_all = npool.tile([128, NG * K], f16, tag="t_all")
                nc.scalar.activation(t_all[:], w_all[:], AF.Exp)
                Zs = spool.tile([128, NG], f32, tag="Zs")
                nc.vector.tensor_reduce(
                    Zs[:], t_all[:].rearrange("p (q k) -> p q k", k=K),
                    axis=AX.X, op=ALU.add)
                recipZ = spool.tile([128, NG], f32, tag="recipZ")
                nc.vector.reciprocal(recipZ[:], Zs[:])
                ss32 = spool.tile([128, NG * K], f32, tag="ss32")
                nc.vector.tensor_tensor(
                    out=ss32[:].rearrange("p (q k) -> p q k", k=K),
                    in0=t_all[:].rearrange("p (q k) -> p q k", k=K),
                    in1=recipZ[:].unsqueeze(2).broadcast_to([128, NG, K]), op=ALU.mult)
                ss_all = npool.tile([128, NG * K], f16, tag="ss_all")
                nc.vector.tensor_copy(ss_all[:], ss32[:])
                tws = spool.tile([128, NG * K], f32, tag="tws")
                nc.vector.tensor_tensor(out=tws[:], in0=t_all[:], in1=w_all[:],
                                        op=ALU.mult)
                Tw = spool.tile([128, NG], f32, tag="Tw")
                nc.vector.tensor_reduce(
                    Tw[:], tws[:].rearrange("p (q k) -> p q k", k=K),
                    axis=AX.X, op=ALU.add)
                lnZ = spool.tile([128, NG], f32, tag="lnZ")
                nc.scalar.activation(lnZ[:], Zs[:], AF.Ln)
                twz = spool.tile([128, NG], f32, tag="twz")
                nc.vector.tensor_tensor(out=twz[:], in0=Tw[:], in1=recipZ[:], op=ALU.mult)
                nc.vector.tensor_tensor(
                    out=S_ent[:, g * NG:(g + 1) * NG], in0=lnZ[:], in1=twz[:],
                    op=ALU.subtract)

                # pass 3: u^T = s^T a ; G = s^T s ; pooled = s^T z
                uT = npool.tile([64, N], f16, tag="uT")
                pG = psm.tile([64, 64], f32, tag="pm")
                ppool = psm.tile([64, 128], f32, tag="pm")
                for h in range(2):
                    pu = pss.tile([64, 512], f32, tag="psmall")
                    for q in range(NG):
                        nc.tensor.matmul(
                            pu[:], ss_all[:, q * K:(q + 1) * K],
                            a_t[q][:, h * 512:(h + 1) * 512],
                            start=(q == 0), stop=(q == NG - 1),
                        )
                    nc.vector.tensor_copy(uT[:, h * 512:(h + 1) * 512], pu[:])
                for q in range(NG):
                    nc.tensor.matmul(pG[:], ss_all[:, q * K:(q + 1) * K],
                                     ss_all[:, q * K:(q + 1) * K],
                                     start=(q == 0), stop=(q == NG - 1))
                for q in range(NG):
                    nc.tensor.matmul(ppool[:], ss_all[:, q * K:(q + 1) * K],
                                     z_all[:, q * 128:(q + 1) * 128],
                                     start=(q == 0), stop=(q == NG - 1))

                gscr = spool.tile([64, 64], f32, tag="gscr")
                nc.scalar.activation(gscr[:], pG[:], AF.Square,
                                     accum_out=S_gsq[:64, g:g + 1])
                pooled = spool.tile([64, 128], f16, tag="pooled")
                nc.vector.tensor_copy(pooled[:], ppool[:])

                u_all = npool.tile([128, NG * K], f16, tag="u_all")
                for q in range(NG):
                    pt = pst.tile([128, 128], f16, tag="tp")
                    nc.tensor.transpose(pt[:, :K], uT[:, q * 128:(q + 1) * 128],
                                        ident[:64, :64])
                    nc.scalar.copy(u_all[:, q * K:(q + 1) * K], pt[:, :K])
                padj = psm.tile([64, 64], f32, tag="pm")
                padjT = psm.tile([64, 64], f32, tag="pm")
                for q in range(NG):
                    nc.tensor.matmul(padj[:], u_all[:, q * K:(q + 1) * K],
                                     ss_all[:, q * K:(q + 1) * K],
                                     start=(q == 0), stop=(q == NG - 1))
                for q in range(NG):
                    nc.tensor.matmul(padjT[:], ss_all[:, q * K:(q + 1) * K],
                                     u_all[:, q * K:(q + 1) * K],
                                     start=(q == 0), stop=(q == NG - 1))

                dscr = spool.tile([64, 64], f32, tag="dscr")
                nc.vector.tensor_tensor(out=dscr[:], in0=padj[:], in1=ident[:64, :64],
                                        op=ALU.mult)
                nc.vector.tensor_reduce(S_diag[:64, g:g + 1], dscr[:], axis=AX.X,
                                        op=ALU.add)
                rs = spool.tile([64, 1], f32, tag="rs")
                nc.vector.tensor_reduce(rs[:], padj[:], axis=AX.X, op=ALU.add)
                nc.vector.tensor_scalar_max(rs[:], rs[:], 1.0)
                nc.vector.reciprocal(rs[:], rs[:])
                adjT = spool.tile([64, 64], f16, tag="adjT")
                nc.vector.tensor_copy(adjT[:], padjT[:])

                pagg3 = psm.tile([64, 128], f32, tag="pm")
                nc.tensor.matmul(pagg3[:], adjT[:], pooled[:])
                agg3n = spool.tile([64, 128], f16, tag="agg3n")
                nc.scalar.activation(agg3n[:], pagg3[:], AF.Copy, scale=rs[:])

                pt3 = pst.tile([128, 128], f16, tag="tp")
                nc.tensor.transpose(pt3[:, :K], agg3n[:], ident[:64, :64])
                agg3nT = spool.tile([128, 64], f16, tag="agg3nT")
                nc.scalar.copy(agg3nT[:], pt3[:, :K])
                pt4 = pst.tile([128, 128], f16, tag="tp")
                nc.tensor.transpose(pt4[:, :K], pooled[:], ident[:64, :64])
                pooledT = spool.tile([128, 64], f16, tag="pooledT")
                nc.scalar.copy(pooledT[:], pt4[:, :K])

                ph1 = psm.tile([128, 64], f32, tag="pm")
                nc.tensor.matmul(ph1[:], Wrel[:], agg3nT[:], start=True, stop=False)
                nc.tensor.matmul(ph1[:], Wroot[:], pooledT[:], start=False, stop=True)
                nc.scalar.activation(Hr[:, :, g], ph1[:], AF.Lrelu, bias=broot[:],
                                     alpha=0.01)

            # ---------------- classifier (all graphs) ----------------
            pcls = psm.tile([128, GPC], f32, tag="pm")
            for k in range(K):
                nc.tensor.matmul(pcls[:], Wc1[k][:], H[:, k * GPC:(k + 1) * GPC],
                                 start=(k == 0), stop=(k == K - 1))
            hc1 = spool.tile([128, GPC], f16, tag="hc1")
            nc.scalar.activation(hc1[:], pcls[:], AF.Lrelu, bias=bc1[:], alpha=0.01)
            plog = psm.tile([1, GPC], f32, tag="pm")
            nc.tensor.matmul(plog[:], Wc2[:], hc1[:])
            lsb = spool.tile([1, GPC], f32, tag="lsb")
            nc.scalar.activation(lsb[:], plog[:], AF.Identity, bias=bc2[:])
            nc.sync.dma_start(logits_out[:], lsb[:])

            # ---------------- loss partial sums ----------------
            Rall = spool.tile([128, 4], f32, tag="Rall")
            nc.vector.tensor_reduce(Rall[:, 0:1], S_ent[:], axis=AX.X, op=ALU.add)
            nc.vector.tensor_reduce(Rall[:, 1:2], S_cnt2[:], axis=AX.X, op=ALU.add)
            nc.vector.tensor_reduce(Rall[:, 2:3], S_diag[:], axis=AX.X, op=ALU.add)
            nc.vector.tensor_reduce(Rall[:, 3:4], S_gsq[:], axis=AX.X, op=ALU.add)
            pfin = psm.tile([4, 1], f32, tag="pm")
            nc.tensor.matmul(pfin[:], Rall[:], ones32[:])
            msb = spool.tile([4, 1], f32, tag="msb")
            nc.vector.tensor_copy(msb[:], pfin[:])
            nc.sync.dma_start(misc_out[:], msb[:])

    nc.compile()
    return nc


def kernel(x, edge_index, batch, num_graphs, nodes_per_graph,
           W1l, b1l, W1r, W2l, b2l, W2r, Wm, bm,
           Wrel, Wroot, broot, Wc1, bc1, Wc2, bc2, **_unused):
    import os
    from concourse.bass_utils import run_bass_kernel_spmd

    x = np.asarray(x, np.float32)
    idxcnt, cntcsc, NI, NJ = _prep_csr(np.asarray(edge_index))

    if (NI, NJ) not in _compiled:
        _compiled[(NI, NJ)] = _build(NI, NJ)
    nc = _compiled[(NI, NJ)]

    def f32c(a, shape=None):
        a = np.ascontiguousarray(np.asarray(a, np.float32))
        return a.reshape(shape) if shape is not None else a

    weights = dict(
        ident=np.eye(128, dtype=np.float16),
        W1l=f32c(W1l), W1r=f32c(W1r), W2l=f32c(W2l), W2r=f32c(W2r),
        Wm=f32c(Wm), Wrel=f32c(Wrel), Wroot=f32c(Wroot), Wc1=f32c(Wc1),
        Wc2=f32c(Wc2, (HID, 1)),
        b1l=f32c(b1l, (HID, 1)), b2l=f32c(b2l, (K, 1)), bm=f32c(bm, (K, 1)),
        broot=f32c(broot, (HID, 1)), bc1=f32c(bc1, (HID, 1)),
        bc2=f32c(bc2, (1, 1)),
    )
    in_maps = []
    for c in range(NCORES):
        in_maps.append(dict(
            xg=np.ascontiguousarray(x[c * GPC * N:(c + 1) * GPC * N]),
            idxcnt=idxcnt[c * GPC:(c + 1) * GPC],
            cntcsc=cntcsc[c * GPC:(c + 1) * GPC],
            **weights,
        ))

    trace = bool(os.environ.get("KERNEL_TRACE"))
    res = run_bass_kernel_spmd(nc, in_maps, list(range(NCORES)), trace=trace)
    kernel.last_results = res

    logits = np.concatenate([res.results[c]["logits"][0] for c in range(NCORES)])
    ent = sum(float(res.results[c]["misc"][0, 0]) for c in range(NCORES))
    cnt2 = sum(float(res.results[c]["misc"][1, 0]) for c in range(NCORES))
    diag = sum(float(res.results[c]["misc"][2, 0]) for c in range(NCORES))
    gsq = sum(float(res.results[c]["misc"][3, 0]) for c in range(NCORES))
    link_sq = max(cnt2 - 2.0 * diag + gsq, 0.0)
    loss = np.float32(np.sqrt(link_sq) / (B * N * N) + ent / (B * N))
    return logits.astype(np.float32), loss


# revision 14
# speedup vs baseline: 1.4818x; 1.0590x over previous
"""Trainium2 Bass kernel for the DiffPool-style GNN (nn_GNNpaper_75368086110729).

Strategy (data-parallel over graphs, 8 graphs/core on 8 cores):
  - Host: convert each graph's edge list to a padded per-source-row CSR
    (unique dst indices + duplicate counts) - pure index/layout prep.
  - Device, per graph:
      * build the dense [1024,1024] adjacency in SBUF as 8 fp16 tiles
        [128 src-partitions x 1024 dst] via the GPSIMD local_scatter ucode op
      * all sparse ops become dense matmuls with the adjacency as the
        PE moving operand: deg = 1^T a, agg1^T = x^T a, agg2^T = z^T a,
        u^T = s^T a (plus pooled = s^T z and G = s^T s riding the same pass)
      * diff-pool losses via  ||a - s s^T||^2 = sum(cnt^2) - 2 tr(adj_p) + ||s^T s||_F^2
      * dense SAGE + classifier on [64,128] tiles
  - Host: concat logits, combine the 4 scalar partial sums
    (loss = sqrt(link_sq)/a.size + ent/(B*N)).
"""

import numpy as np

B, N, E_PER = 64, 1024, 32768
IN_DIM, HID, K = 128, 128, 64
NCORES = 8
GPC = B // NCORES  # graphs per core
NG = N // 128      # 128-row groups per graph

_compiled = {}


def _prep_csr(edge_index):
    """edge_index [2, B*E_PER] int32 -> [B, NG, 2, 128, NI] int16 packed
    (t=0: unique dst idx, -1 pad; t=1: fp16 duplicate counts bit-cast)."""
    src = np.asarray(edge_index[0], dtype=np.int64)
    dst = np.asarray(edge_index[1], dtype=np.int64)
    g = src >> 10
    s = src & 1023
    d = dst & 1023
    key = (g << 20) | (s << 10) | d
    uk, cnts = np.unique(key, return_counts=True)
    rows = (uk >> 10).astype(np.int64)
    dloc = (uk & 1023).astype(np.int16)
    rowlen = np.bincount(rows, minlength=B * N)
    NI = int(max(2, ((rowlen.max() + 1) // 2) * 2))
    row_starts = np.zeros(B * N, np.int64)
    np.cumsum(rowlen[:-1], out=row_starts[1:])
    pos = np.arange(len(uk)) - row_starts[rows]
    idx = np.full((B * N, NI), -1, np.int16)
    cnt = np.zeros((B * N, NI), np.float16)
    idx[rows, pos] = dloc
    cnt[rows, pos] = cnts.astype(np.float16)
    packed = np.stack([idx.reshape(B, NG, 128, NI),
                       cnt.view(np.int16).reshape(B, NG, 128, NI)], axis=2)

    # CSC-ordered counts (per-dst rows) - lets the device compute in-degrees
    # as cheap free-dim row sums in the transposed [128, NG] layout.
    key2 = (g << 20) | (d << 10) | s
    uk2, cnts2 = np.unique(key2, return_counts=True)
    rows2 = (uk2 >> 10).astype(np.int64)
    rowlen2 = np.bincount(rows2, minlength=B * N)
    NJ = int(max(2, ((rowlen2.max() + 1) // 2) * 2))
    row_starts2 = np.zeros(B * N, np.int64)
    np.cumsum(rowlen2[:-1], out=row_starts2[1:])
    pos2 = np.arange(len(uk2)) - row_starts2[rows2]
    cntcsc = np.zeros((B * N, NJ), np.float16)
    cntcsc[rows2, pos2] = cnts2.astype(np.float16)
    cntcsc = np.ascontiguousarray(cntcsc.reshape(B, NG, 128, NJ))
    return np.ascontiguousarray(packed), cntcsc, NI, NJ


def _build(NI, NJ):
    import concourse.bacc as bacc
    import concourse.mybir as mybir
    from concourse import tile, library_config

    f16 = mybir.dt.float16
    f32 = mybir.dt.float32
    i16 = mybir.dt.int16
    AF = mybir.ActivationFunctionType
    ALU = mybir.AluOpType
    AX = mybir.AxisListType

    nc = bacc.Bacc("TRN2", target_bir_lowering=False, debug=False)

    def din(name, shape, dt=f32):
        return nc.dram_tensor(name, shape, dt, kind="ExternalInput").ap()

    xg = din("xg", [GPC * N, IN_DIM])
    idxcnt = din("idxcnt", [GPC, NG, 2, 128, NI], i16)
    cntcsc_in = din("cntcsc", [GPC, NG, 128, NJ], f16)
    ident_in = din("ident", [128, 128], f16)
    W1l_in = din("W1l", [IN_DIM, HID])
    W1r_in = din("W1r", [IN_DIM, HID])
    W2l_in = din("W2l", [HID, K])
    W2r_in = din("W2r", [HID, K])
    Wm_in = din("Wm", [K, K])
    Wrel_in = din("Wrel", [HID, HID])
    Wroot_in = din("Wroot", [HID, HID])
    Wc1_in = din("Wc1", [K * HID, HID])
    Wc2_in = din("Wc2", [HID, 1])
    b1l_in = din("b1l", [HID, 1])
    b2l_in = din("b2l", [K, 1])
    bm_in = din("bm", [K, 1])
    broot_in = din("broot", [HID, 1])
    bc1_in = din("bc1", [HID, 1])
    bc2_in = din("bc2", [1, 1])

    logits_out = nc.dram_tensor("logits", [1, GPC], f32, kind="ExternalOutput").ap()
    misc_out = nc.dram_tensor("misc", [4, 1], f32, kind="ExternalOutput").ap()

    with tile.TileContext(nc) as tc:
        nc.gpsimd.load_library(library_config.local_scatter)
        with (
            tc.tile_pool(name="const", bufs=1) as cpool,
            tc.tile_pool(name="a", bufs=3) as apool,
            tc.tile_pool(name="io", bufs=4) as iopool,
            tc.tile_pool(name="nodes", bufs=3) as npool,
            tc.tile_pool(name="small", bufs=3) as spool,
            tc.tile_pool(name="psb", bufs=2, space="PSUM") as psb,   # [128,512] f32
            tc.tile_pool(name="pss", bufs=2, space="PSUM") as pss,   # [64,512] f32
            tc.tile_pool(name="pst", bufs=2, space="PSUM") as pst,   # [128,128] f16
            tc.tile_pool(name="psm", bufs=2, space="PSUM") as psm,   # small f32
        ):
            # ---------------- constants / weights ----------------
            ident = cpool.tile([128, 128], f16)
            nc.sync.dma_start(ident[:], ident_in[:])
            ones_col = cpool.tile([128, 1], f16)
            nc.vector.memset(ones_col[:], 1.0)
            ones_row = cpool.tile([1, 128], f16)
            nc.vector.memset(ones_row[:], 1.0)
            ones32 = cpool.tile([128, 1], f32)
            nc.vector.memset(ones32[:], 1.0)

            def wtile(ap_in, p, q, name):
                t32 = iopool.tile([p, q], f32, tag="w32")
                nc.sync.dma_start(t32[:], ap_in[:])
                t = cpool.tile([p, q], f16, tag=name)
                nc.vector.tensor_copy(t[:], t32[:])
                return t

            W1l = wtile(W1l_in, 128, 128, "W1l")
            W1r = wtile(W1r_in, 128, 128, "W1r")
            W2l = wtile(W2l_in, 128, 64, "W2l")
            W2r = wtile(W2r_in, 128, 64, "W2r")
            Wm = wtile(Wm_in, 64, 64, "Wm")
            Wrel = wtile(Wrel_in, 128, 128, "Wrel")
            Wroot = wtile(Wroot_in, 128, 128, "Wroot")
            Wc2 = wtile(Wc2_in, 128, 1, "Wc2")

            Wc1 = []
            for k in range(K):
                t32 = iopool.tile([128, 128], f32, tag="wc132")
                nc.sync.dma_start(t32[:], Wc1_in[k * 128:(k + 1) * 128, :])
                t = cpool.tile([128, 128], f16, tag=f"wc1_{k}")
                nc.vector.tensor_copy(t[:], t32[:])
                Wc1.append(t)

            def btile(ap_in, p, name):
                t = cpool.tile([p, 1], f32, tag=name)
                nc.sync.dma_start(t[:], ap_in[:])
                return t

            b1l = btile(b1l_in, 128, "b1l")
            b2l = btile(b2l_in, 64, "b2l")
            bm = btile(bm_in, 64, "bm")
            broot = btile(broot_in, 128, "broot")
            bc1 = btile(bc1_in, 128, "bc1")
            bc2 = btile(bc2_in, 1, "bc2")

            # loss accumulators
            S_ent = cpool.tile([128, B], f32)
            S_cnt2 = cpool.tile([128, GPC], f32)
            S_diag = cpool.tile([128, GPC], f32)
            S_gsq = cpool.tile([128, GPC], f32)
            for t in (S_ent, S_cnt2, S_diag, S_gsq):
                nc.vector.memset(t[:], 0.0)
            H = cpool.tile([128, GPC * K], f16)  # col k*GPC+g
            Hr = H[:].rearrange("p (k g) -> p k g", g=GPC)

            # ---------------- per-graph pipeline ----------------
            for g in range(GPC):
                # CSR load (one DMA), adjacency build, sum(cnt^2)
                icb = iopool.tile([128, NG, 2, NI], i16, tag="icb")
                nc.sync.dma_start(icb[:], idxcnt[g].rearrange("q t p i -> p q t i"))
                a_t = []
                for q in range(NG):
                    at = apool.tile([128, N], f16, tag=f"a{q}")
                    nc.gpsimd.local_scatter(
                        at[:], icb[:, q, 1, :].bitcast(f16), icb[:, q, 0, :],
                        channels=128, num_elems=N, num_idxs=NI,
                    )
                    a_t.append(at)
                cscr = iopool.tile([128, NG * NI], f16, tag="cscr")
                nc.scalar.activation(
                    cscr[:].rearrange("p (q i) -> p q i", i=NI),
                    icb[:, :, 1, :].bitcast(f16), AF.Square,
                    accum_out=S_cnt2[:, g:g + 1],
                )

                # x load + cast + transposed copy
                x32 = iopool.tile([128, N], f32, tag="x32")
                nc.scalar.dma_start(
                    x32[:].rearrange("p (q f) -> p q f", q=NG),
                    xg[g * N:(g + 1) * N, :].rearrange("(q p) f -> p q f", p=128))
                x16 = npool.tile([128, N], f16, tag="x16")
                nc.vector.tensor_copy(x16[:], x32[:])
                xT = npool.tile([128, N], f16, tag="xT")
                for q in range(NG):
                    pt = pst.tile([128, 128], f16, tag="tp")
                    nc.tensor.transpose(pt[:], x16[:, q * 128:(q + 1) * 128], ident[:])
                    nc.vector.tensor_copy(xT[:, q * 128:(q + 1) * 128], pt[:])

                # deg from CSC count row-sums -> [128, NG] (partition = dst%128)
                ccs = iopool.tile([128, NG * NJ], f16, tag="ccs")
                nc.scalar.dma_start(
                    ccs[:].rearrange("p (q j) -> p q j", j=NJ),
                    cntcsc_in[g].rearrange("q p j -> p q j"))
                deg_t = spool.tile([128, NG], f32, tag="deg_t")
                nc.vector.tensor_reduce(
                    deg_t[:], ccs[:].rearrange("p (q j) -> p q j", j=NJ),
                    axis=AX.X, op=ALU.add)
                nc.vector.tensor_scalar_max(deg_t[:], deg_t[:], 1.0)
                rcp_t = spool.tile([128, NG], f32, tag="rcp_t")
                nc.vector.reciprocal_approx_fast(rcp_t[:], deg_t[:])
                rcp16 = spool.tile([128, NG], f16, tag="rcp16")
                nc.vector.tensor_copy(rcp16[:], rcp_t[:])
                ptr = pst.tile([128, 128], f16, tag="tp")
                nc.tensor.transpose(ptr[:NG, :], rcp16[:], ident[:])
                rcpT = spool.tile([NG, 128], f16, tag="rcpT")
                nc.vector.tensor_copy(rcpT[:], ptr[:NG, :])
                recipb = npool.tile([128, N], f16, tag="recipb")
                for q in range(NG):
                    rq = spool.tile([1, 128], f16, tag=f"rcprow{q}")
                    nc.scalar.dma_start(rq[:], rcpT[q:q + 1, :])
                    pb = pst.tile([128, 128], f32, tag="tp")
                    nc.tensor.matmul(pb[:], ones_row[:], rq[:])
                    nc.vector.tensor_copy(recipb[:, q * 128:(q + 1) * 128], pb[:])

                # pass 1: agg1^T = x^T a ; normalize
                aggn1T = npool.tile([128, N], f16, tag="aggn1")
                for h in range(2):
                    p1 = psb.tile([128, 512], f32, tag="pbig")
                    for q in range(NG):
                        nc.tensor.matmul(
                            p1[:], x16[:, q * 128:(q + 1) * 128],
                            a_t[q][:, h * 512:(h + 1) * 512],
                            start=(q == 0), stop=(q == NG - 1),
                        )
                    nc.vector.tensor_tensor(
                        out=aggn1T[:, h * 512:(h + 1) * 512], in0=p1[:],
                        in1=recipb[:, h * 512:(h + 1) * 512], op=ALU.mult,
                    )

                # z^T = lrelu(W1l^T aggn1T + W1r^T xT + b1l)
                zT = npool.tile([128, N], f16, tag="zT")
                for h in range(2):
                    pz = psb.tile([128, 512], f32, tag="pbig")
                    nc.tensor.matmul(pz[:], W1l[:], aggn1T[:, h * 512:(h + 1) * 512],
                                     start=True, stop=False)
                    nc.tensor.matmul(pz[:], W1r[:], xT[:, h * 512:(h + 1) * 512],
                                     start=False, stop=True)
                    nc.scalar.activation(zT[:, h * 512:(h + 1) * 512], pz[:],
                                         AF.Lrelu, bias=b1l[:], alpha=0.01)
                z_all = npool.tile([128, N], f16, tag="z_all")
                for q in range(NG):
                    pt = pst.tile([128, 128], f16, tag="tp")
                    nc.tensor.transpose(pt[:], zT[:, q * 128:(q + 1) * 128], ident[:])
                    nc.vector.tensor_copy(z_all[:, q * 128:(q + 1) * 128], pt[:])

                # pass 2: agg2^T = z^T a ; normalize
                aggn2T = npool.tile([128, N], f16, tag="aggn2")
                for h in range(2):
                    p2 = psb.tile([128, 512], f32, tag="pbig")
                    for q in range(NG):
                        nc.tensor.matmul(
                            p2[:], z_all[:, q * 128:(q + 1) * 128],
                            a_t[q][:, h * 512:(h + 1) * 512],
                            start=(q == 0), stop=(q == NG - 1),
                        )
                    nc.vector.tensor_tensor(
                        out=aggn2T[:, h * 512:(h + 1) * 512], in0=p2[:],
                        in1=recipb[:, h * 512:(h + 1) * 512], op=ALU.mult,
                    )

                # s1^T = lrelu(W2l^T aggn2T + W2r^T zT + b2l); s2^T = Wm^T s1T + bm
                s1T = npool.tile([64, N], f16, tag="s1T")
                for h in range(2):
                    ps1 = pss.tile([64, 512], f32, tag="psmall")
                    nc.tensor.matmul(ps1[:], W2l[:], aggn2T[:, h * 512:(h + 1) * 512],
                                     start=True, stop=False)
                    nc.tensor.matmul(ps1[:], W2r[:], zT[:, h * 512:(h + 1) * 512],
                                     start=False, stop=True)
                    nc.scalar.activation(s1T[:, h * 512:(h + 1) * 512], ps1[:],
                                         AF.Lrelu, bias=b2l[:], alpha=0.01)
                s2T = npool.tile([64, N], f16, tag="s2T")
                for h in range(2):
                    ps2 = pss.tile([64, 512], f32, tag="psmall")
                    nc.tensor.matmul(ps2[:], Wm[:], s1T[:, h * 512:(h + 1) * 512])
                    nc.scalar.activation(s2T[:, h * 512:(h + 1) * 512], ps2[:],
                                         AF.Identity, bias=bm[:])
                s2_all = npool.tile([128, NG * K], f16, tag="s2_all")
                for q in range(NG):
                    pt = pst.tile([128, 128], f16, tag="tp")
                    nc.tensor.transpose(pt[:, :K], s2T[:, q * 128:(q + 1) * 128],
                                        ident[:64, :64])
                    nc.scalar.copy(s2_all[:, q * K:(q + 1) * K], pt[:, :K])

                # batched softmax over k + entropy pieces
                s2r = s2_all[:].rearrange("p (q k) -> p q k", k=K)
                Ms = spool.tile([128, NG], f32, tag="Ms")
                nc.vector.tensor_reduce(Ms[:], s2r, axis=AX.X, op=ALU.max)
                w_all = spool.tile([128, NG * K], f32, tag="w_all")
                nc.vector.tensor_tensor(
                    out=w_all[:].rearrange("p (q k) -> p q k", k=K), in0=s2r,
                    in1=Ms[:].unsqueeze(2).broadcast_to([128, NG, K]), op=ALU.subtract)
                t_all = npool.tile([128, NG * K], f16, tag="t_all")
                nc.scalar.activation(t_all[:], w_all[:], AF.Exp)
                Zs = spool.tile([128, NG], f32, tag="Zs")
                nc.vector.tensor_reduce(
                    Zs[:], t_all[:].rearrange("p (q k) -> p q k", k=K),
                    axis=AX.X, op=ALU.add)
                recipZ = spool.tile([128, NG], f32, tag="recipZ")
                nc.vector.reciprocal(recipZ[:], Zs[:])
                ss32 = spool.tile([128, NG * K], f32, tag="ss32")
                nc.vector.tensor_tensor(
                    out=ss32[:].rearrange("p (q k) -> p q k", k=K),
                    in0=t_all[:].rearrange("p (q k) -> p q k", k=K),
                    in1=recipZ[:].unsqueeze(2).broadcast_to([128, NG, K]), op=ALU.mult)
                ss_all = npool.tile([128, NG * K], f16, tag="ss_all")
                nc.vector.tensor_copy(ss_all[:], ss32[:])
                tws = spool.tile([128, NG * K], f32, tag="tws")
                nc.vector.tensor_tensor(out=tws[:], in0=t_all[:], in1=w_all[:],
                                        op=ALU.mult)
                Tw = spool.tile([128, NG], f32, tag="Tw")
                nc.vector.tensor_reduce(
                    Tw[:], tws[:].rearrange("p (q k) -> p q k", k=K),
                    axis=AX.X, op=ALU.add)
                lnZ = spool.tile([128, NG], f32, tag="lnZ")
                nc.scalar.activation(lnZ[:], Zs[:], AF.Ln)
                twz = spool.tile([128, NG], f32, tag="twz")
                nc.vector.tensor_tensor(out=twz[:], in0=Tw[:], in1=recipZ[:], op=ALU.mult)
                nc.vector.tensor_tensor(
                    out=S_ent[:, g * NG:(g + 1) * NG], in0=lnZ[:], in1=twz[:],
                    op=ALU.subtract)

                # pass 3: u^T = s^T a ; G = s^T s ; pooled = s^T z
                uT = npool.tile([64, N], f16, tag="uT")
                pG = psm.tile([64, 64], f32, tag="pm")
                ppool = psm.tile([64, 128], f32, tag="pm")
                for h in range(2):
                    pu = pss.tile([64, 512], f32, tag="psmall")
                    for q in range(NG):
                        nc.tensor.matmul(
                            pu[:], ss_all[:, q * K:(q + 1) * K],
                            a_t[q][:, h * 512:(h + 1) * 512],
                            start=(q == 0), stop=(q == NG - 1),
                        )
                    nc.vector.tensor_copy(uT[:, h * 512:(h + 1) * 512], pu[:])
                for q in range(NG):
                    nc.tensor.matmul(pG[:], ss_all[:, q * K:(q + 1) * K],
                                     ss_all[:, q * K:(q + 1) * K],
                                     start=(q == 0), stop=(q == NG - 1))
                for q in range(NG):
                    nc.tensor.matmul(ppool[:], ss_all[:, q * K:(q + 1) * K],
                                     z_all[:, q * 128:(q + 1) * 128],
                                     start=(q == 0), stop=(q == NG - 1))

                gscr = spool.tile([64, 64], f32, tag="gscr")
                nc.scalar.activation(gscr[:], pG[:], AF.Square,
                                     accum_out=S_gsq[:64, g:g + 1])
                pooled = spool.tile([64, 128], f16, tag="pooled")
                nc.vector.tensor_copy(pooled[:], ppool[:])

                u_all = npool.tile([128, NG * K], f16, tag="u_all")
                for q in range(NG):
                    pt = pst.tile([128, 128], f16, tag="tp")
                    nc.tensor.transpose(pt[:, :K], uT[:, q * 128:(q + 1) * 128],
                                        ident[:64, :64])
                    nc.scalar.copy(u_all[:, q * K:(q + 1) * K], pt[:, :K])
                padj = psm.tile([64, 64], f32, tag="pm")
                padjT = psm.tile([64, 64], f32, tag="pm")
                for q in range(NG):
                    nc.tensor.matmul(padj[:], u_all[:, q * K:(q + 1) * K],
                                     ss_all[:, q * K:(q + 1) * K],
                                     start=(q == 0), stop=(q == NG - 1))
                for q in range(NG):
                    nc.tensor.matmul(padjT[:], ss_all[:, q * K:(q + 1) * K],
                                     u_all[:, q * K:(q + 1) * K],
                                     start=(q == 0), stop=(q == NG - 1))

                dscr = spool.tile([64, 64], f32, tag="dscr")
                nc.vector.tensor_tensor(out=dscr[:], in0=padj[:], in1=ident[:64, :64],
                                        op=ALU.mult)
                nc.vector.tensor_reduce(S_diag[:64, g:g + 1], dscr[:], axis=AX.X,
                                        op=ALU.add)
                rs = spool.tile([64, 1], f32, tag="rs")
                nc.vector.tensor_reduce(rs[:], padj[:], axis=AX.X, op=ALU.add)
                nc.vector.tensor_scalar_max(rs[:], rs[:], 1.0)
                nc.vector.reciprocal(rs[:], rs[:])
                adjT = spool.tile([64, 64], f16, tag="adjT")
                nc.vector.tensor_copy(adjT[:], padjT[:])

                pagg3 = psm.tile([64, 128], f32, tag="pm")
                nc.tensor.matmul(pagg3[:], adjT[:], pooled[:])
                agg3n = spool.tile([64, 128], f16, tag="agg3n")
                nc.scalar.activation(agg3n[:], pagg3[:], AF.Copy, scale=rs[:])

                pt3 = pst.tile([128, 128], f16, tag="tp")
                nc.tensor.transpose(pt3[:, :K], agg3n[:], ident[:64, :64])
                agg3nT = spool.tile([128, 64], f16, tag="agg3nT")
                nc.scalar.copy(agg3nT[:], pt3[:, :K])
                pt4 = pst.tile([128, 128], f16, tag="tp")
                nc.tensor.transpose(pt4[:, :K], pooled[:], ident[:64, :64])
                pooledT = spool.tile([128, 64], f16, tag="pooledT")
                nc.scalar.copy(pooledT[:], pt4[:, :K])

                ph1 = psm.tile([128, 64], f32, tag="pm")
                nc.tensor.matmul(ph1[:], Wrel[:], agg3nT[:], start=True, stop=False)
                nc.tensor.matmul(ph1[:], Wroot[:], pooledT[:], start=False, stop=True)
                nc.scalar.activation(Hr[:, :, g], ph1[:], AF.Lrelu, bias=broot[:],
                                     alpha=0.01)

            # ---------------- classifier (all graphs) ----------------
            pcls = psm.tile([128, GPC], f32, tag="pm")
            for k in range(K):
                nc.tensor.matmul(pcls[:], Wc1[k][:], H[:, k * GPC:(k + 1) * GPC],
                                 start=(k == 0), stop=(k == K - 1))
            hc1 = spool.tile([128, GPC], f16, tag="hc1")
            nc.scalar.activation(hc1[:], pcls[:], AF.Lrelu, bias=bc1[:], alpha=0.01)
            plog = psm.tile([1, GPC], f32, tag="pm")
            nc.tensor.matmul(plog[:], Wc2[:], hc1[:])
            lsb = spool.tile([1, GPC], f32, tag="lsb")
            nc.scalar.activation(lsb[:], plog[:], AF.Identity, bias=bc2[:])
            nc.sync.dma_start(logits_out[:], lsb[:])

            # ---------------- loss partial sums ----------------
            Rall = spool.tile([128, 4], f32, tag="Rall")
            nc.vector.tensor_reduce(Rall[:, 0:1], S_ent[:], axis=AX.X, op=ALU.add)
            nc.vector.tensor_reduce(Rall[:, 1:2], S_cnt2[:], axis=AX.X, op=ALU.add)
            nc.vector.tensor_reduce(Rall[:, 2:3], S_diag[:], axis=AX.X, op=ALU.add)
            nc.vector.tensor_reduce(Rall[:, 3:4], S_gsq[:], axis=AX.X, op=ALU.add)
            pfin = psm.tile([4, 1], f32, tag="pm")
            nc.tensor.matmul(pfin[:], Rall[:], ones32[:])
            msb = spool.tile([4, 1], f32, tag="msb")
            nc.vector.tensor_copy(msb[:], pfin[:])
            nc.sync.dma_start(misc_out[:], msb[:])

    nc.compile()
    return nc


def kernel(x, edge_index, batch, num_graphs, nodes_per_graph,
           W1l, b1l, W1r, W2l, b2l, W2r, Wm, bm,
           Wrel, Wroot, broot, Wc1, bc1, Wc2, bc2, **_unused):
    import os
    from concourse.bass_utils import run_bass_kernel_spmd

    x = np.asarray(x, np.float32)
    idxcnt, cntcsc, NI, NJ = _prep_csr(np.asarray(edge_index))

    if (NI, NJ) not in _compiled:
        _compiled[(NI, NJ)] = _build(NI, NJ)
    nc = _compiled[(NI, NJ)]

    def f32c(a, shape=None):
        a = np.ascontiguousarray(np.asarray(a, np.float32))
        return a.reshape(shape) if shape is not None else a

    weights = dict(
        ident=np.eye(128, dtype=np.float16),
        W1l=f32c(W1l), W1r=f32c(W1r), W2l=f32c(W2l), W2r=f32c(W2r),
        Wm=f32c(Wm), Wrel=f32c(Wrel), Wroot=f32c(Wroot), Wc1=f32c(Wc1),
        Wc2=f32c(Wc2, (HID, 1)),
        b1l=f32c(b1l, (HID, 1)), b2l=f32c(b2l, (K, 1)), bm=f32c(bm, (K, 1)),
        broot=f32c(broot, (HID, 1)), bc1=f32c(bc1, (HID, 1)),
        bc2=f32c(bc2, (1, 1)),
    )
    in_maps = []
    for c in range(NCORES):
        in_maps.append(dict(
            xg=np.ascontiguousarray(x[c * GPC * N:(c + 1) * GPC * N]),
            idxcnt=idxcnt[c * GPC:(c + 1) * GPC],
            cntcsc=cntcsc[c * GPC:(c + 1) * GPC],
            **weights,
        ))

    trace = bool(os.environ.get("KERNEL_TRACE"))
    res = run_bass_kernel_spmd(nc, in_maps, list(range(NCORES)), trace=trace)
    kernel.last_results = res

    logits = np.concatenate([res.results[c]["logits"][0] for c in range(NCORES)])
    ent = sum(float(res.results[c]["misc"][0, 0]) for c in range(NCORES))
    cnt2 = sum(float(res.results[c]["misc"][1, 0]) for c in range(NCORES))
    diag = sum(float(res.results[c]["misc"][2, 0]) for c in range(NCORES))
    gsq = sum(float(res.results[c]["misc"][3, 0]) for c in range(NCORES))
    link_sq = max(cnt2 - 2.0 * diag + gsq, 0.0)
    loss = np.float32(np.sqrt(link_sq) / (B * N * N) + ent / (B * N))
    return logits.astype(np.float32), loss
